# revision 66
# baseline (speedup 1.0000x reference)
"""Trainium2 Bass kernel for nn_MultiHeadAttention_63015760167496.

Computation (see reference): qkv = x @ Wqkv; RoPE on q,k; causal softmax
attention per head; out = einsum('bhts,bshd->bhtd', probs, v);
out.reshape(B,T,C) @ Wout  -- NOTE the reshape is a *head-major* flatten of
[B,H,T,D] into [B,T,C], so final-output row r = h*128 + t//16 depends only on
head h.  Sharding: head-parallel over 8 cores (2 heads/core); every core
computes its two heads end-to-end and produces final-output rows
[256*i, 256*i+256).  Host concatenates -- no collectives.

v3: all matmul payloads bf16 (host-cast); 512-wide tiles; the whole kernel is
one continuous PE instruction stream with cross-phase software pipelining:
  P1 = qkv(b0)
  P2 = attention(b0) interleaved with qkv(b1)
  P3 = attention(b1) interleaved with out-projection of b0's heads
  P4 = out-projection of b1's heads
The interleave keeps the tensor engine busy back-to-back (sustaining the
2.4 GHz p-state) and hides the scalar-engine exp + the softmax-normalize
dependency chains behind independent GEMM groups.  Attention is computed in
S^T layout ([s,t]): denominator via a ones-vector matmul, normalization via a
K=1 broadcast matmul; AV/sum flushes lag their scores chunk by 2 slots and
each tile's normalize lags 3 more items.  PSUM is four top-level rings shared
by all phases.
"""

import math
import sys

for _p in ("/opt/trn_rl_repo", "/root/.axon_site/_ro/trn_rl_repo"):
    if _p not in sys.path:
        sys.path.insert(0, _p)

import numpy as np
import ml_dtypes

import concourse.bass as bass
import concourse.mybir as mybir
import concourse.tile as tile
from concourse import bacc
from concourse.bass_utils import run_bass_kernel_spmd

B, T, C = 2, 2048, 2048
H = 16            # heads total
D = C // H        # 128 head dim
HALF = D // 2     # 64
P = 128
KO = C // P       # 16 contraction chunks
KG = 4            # ko chunks per startup DMA group
NG = KO // KG     # 4 startup groups
NCORES = 8
HPC = H // NCORES  # 2 heads per core
TQ = 512          # t-tile for qkv projection
NT = T // TQ      # 4
TA = 512          # t-tile for attention
NTA = T // TA     # 4
NSC = T // P      # 16 s-chunks
TC = 512          # col-tile for out projection
NTC = C // TC     # 4
ROPE_BASE = 10000.0
SCALE = 1.0 / math.sqrt(D)

f32 = mybir.dt.float32
f32r = mybir.dt.float32r
bf16 = mybir.dt.bfloat16
npbf16 = ml_dtypes.bfloat16


def _build():
    nc = bacc.Bacc("TRN2", target_bir_lowering=False, debug=False,
                   num_devices=NCORES)

    # host-pre-tiled x^T: xTt[b, ti, p, ko, u] = x[b, ti*TQ+u, ko*128+p]
    xTt = nc.dram_tensor("xTt", [B, NT, P, KO, TQ], bf16, kind="ExternalInput")
    # host-pre-chunked weights: w[p, ko, m] = W[ko*128+p, m]
    wq = nc.dram_tensor("wq", [P, KO, HPC * D], bf16, kind="ExternalInput")
    wk = nc.dram_tensor("wk", [P, KO, HPC * D], bf16, kind="ExternalInput")
    wv = nc.dram_tensor("wv", [P, KO, HPC * D], bf16, kind="ExternalInput")
    # wout4[cpi, p, j, u] = Wout[j*128+p, cpi*TC+u]
    wout = nc.dram_tensor("wout", [NTC, P, KO, TC], bf16, kind="ExternalInput")
    cs2 = nc.dram_tensor("cs2", [P, T], bf16, kind="ExternalInput")  # [cos;cos]
    sn1 = nc.dram_tensor("sn1", [HALF, T], bf16, kind="ExternalInput")  # sin
    maskM = nc.dram_tensor("maskM", [P, P], bf16, kind="ExternalInput")
    y = nc.dram_tensor("y", [B, HPC * D, C], f32, kind="ExternalOutput")

    with tile.TileContext(nc) as tc:
        with tc.tile_pool(name="const", bufs=1) as cp_, \
             tc.tile_pool(name="qkv", bufs=1) as qp, \
             tc.tile_pool(name="ot", bufs=1) as op_, \
             tc.tile_pool(name="xt", bufs=2) as xp, \
             tc.tile_pool(name="wcp", bufs=2) as wp, \
             tc.tile_pool(name="rope", bufs=2) as rp, \
             tc.tile_pool(name="pt", bufs=8) as ptp, \
             tc.tile_pool(name="small", bufs=2) as sp, \
             tc.tile_pool(name="psqk", bufs=3, space="PSUM") as ring_qk, \
             tc.tile_pool(name="pssc", bufs=2, space="PSUM") as ring_sc, \
             tc.tile_pool(name="psacc", bufs=2, space="PSUM") as ring_acc, \
             tc.tile_pool(name="pssum", bufs=1, space="PSUM") as ring_sum:

            wq_g = [cp_.tile([P, KG, HPC * D], bf16, tag=f"wqg{g}",
                             name=f"wqg{g}") for g in range(NG)]
            wk_sb = cp_.tile([P, KO, HPC * D], bf16, tag="wk")
            wv_sb = cp_.tile([P, KO, HPC * D], bf16, tag="wv")
            cs_sb = cp_.tile([P, T], bf16, tag="cs")
            sn_sb = cp_.tile([HALF, T], bf16, tag="sn")
            mask_sb = cp_.tile([P, P], bf16, tag="mask")

            ones_f32 = cp_.tile([P, 1], f32, tag="ones_f32")
            nc.vector.memset(ones_f32[:], 1.0)
            ones_col = cp_.tile([P, 1], bf16, tag="ones_col")
            nc.vector.tensor_copy(ones_col[:], ones_f32[:])
            onesr_f32 = cp_.tile([1, P], f32, tag="onesr_f32")
            nc.vector.memset(onesr_f32[:], 1.0)
            ones_row = cp_.tile([1, P], f32r, tag="ones_row")
            nc.vector.tensor_copy(ones_row[:], onesr_f32[:])

            # startup DMAs: issue from four parallel engine queues (the sync
            # sequencer takes ~650ns per issue — serializing all of these
            # on one queue costs ~6us before the last transfer even starts).
            # wq on sync, xt00 on vector, wk/wv on scalar, cos/sin on gpsimd.
            xt00 = xp.tile([P, KO, TQ], bf16, tag="xt", name="xt00")
            for g in range(NG):
                ksl = slice(g * KG, (g + 1) * KG)
                nc.sync.dma_start(wq_g[g][:], wq.ap()[:, ksl])
                nc.gpsimd.dma_start(xt00[:, ksl, :], xTt.ap()[0, 0, :, ksl])
            nc.scalar.dma_start(wk_sb[:, 0:KO // 2], wk.ap()[:, 0:KO // 2])
            nc.scalar.dma_start(wk_sb[:, KO // 2:KO], wk.ap()[:, KO // 2:KO])
            nc.gpsimd.dma_start(cs_sb[:], cs2.ap())
            nc.gpsimd.dma_start(sn_sb[:], sn1.ap())
            nc.scalar.dma_start(wv_sb[:, 0:KO // 2], wv.ap()[:, 0:KO // 2])
            nc.scalar.dma_start(wv_sb[:, KO // 2:KO], wv.ap()[:, KO // 2:KO])
            nc.sync.dma_start(mask_sb[:], maskM.ap())

            # remaining x tiles; WAR on the pool ring paces the sync queue
            xts = {(0, 0): xt00}
            for b in range(B):
                for ti in range(NT):
                    if b == 0 and ti == 0:
                        continue
                    xt = xp.tile([P, KO, TQ], bf16, tag="xt",
                                 name=f"xt{b}{ti}")
                    nc.sync.dma_start(xt[:], xTt.ap()[b, ti])
                    xts[(b, ti)] = xt
            # wout slices: one copy per consuming half-pass (b0 pass in P3,
            # b1 pass in P4) so the ring-2 WAR chain stays acyclic.
            # two out-projection passes (b0 both heads, then b1), each with
            # its own wout copy so the ring-2 WAR chain stays acyclic
            # half-size tiles in a ring of 4 (same SBUF footprint as two
            # full tiles): each WAR-gated DMA is 1MB, landing sooner
            wcs = {}
            for half in range(2):
                for cpi in range(NTC):
                    for part in range(2):
                        w = wp.tile([P, KO // 2, TC], bf16, tag="wc", bufs=5,
                                    name=f"wc{half}_{cpi}_{part}")
                        for q4 in range(2):
                            jsl = slice(q4 * KO // 4, (q4 + 1) * KO // 4)
                            gsl = slice(part * KO // 2 + q4 * KO // 4,
                                        part * KO // 2 + (q4 + 1) * KO // 4)
                            nc.sync.dma_start(w[:, jsl, :],
                                              wout.ap()[cpi, :, gsl])
                        wcs[(half, cpi, part)] = w

            # persistent tensors per (b, local head)
            oT = [[op_.tile([P, T], bf16, tag=f"oT{b}{hh}", name=f"oT{b}{hh}")
                   for hh in range(HPC)] for b in range(B)]
            qT = [[qp.tile([P, T], bf16, tag=f"qT{b}{hh}", name=f"qT{b}{hh}")
                   for hh in range(HPC)] for b in range(B)]
            kT = [[qp.tile([P, T], bf16, tag=f"kT{b}{hh}", name=f"kT{b}{hh}")
                   for hh in range(HPC)] for b in range(B)]
            vt = [[qp.tile([P, NSC, D], bf16, tag=f"v{b}{hh}",
                           name=f"v{b}{hh}") for hh in range(HPC)]
                  for b in range(B)]

            # ---------------- emitters ----------------
            def rope(ps, dst, sl, cs, sn):
                # tcos = ps * [cos;cos]; tsw pre-swaps halves (tsw[0:64] =
                # q2*sin, tsw[64:128] = q1*sin) so reads align on base
                # partitions; muls on DVE (psum-capable), sub/add on gpsimd.
                tcos = rp.tile([P, TQ], f32, tag="tcos", name="tcos")
                tsw = rp.tile([P, TQ], f32, tag="tsw", name="tsw")
                nc.vector.tensor_mul(tcos[:], ps[:], cs)
                nc.vector.tensor_mul(tsw[0:HALF, :], ps[HALF:P, :], sn)
                nc.vector.tensor_mul(tsw[HALF:P, :], ps[0:HALF, :], sn)
                nc.gpsimd.tensor_sub(dst[0:HALF, sl], tcos[0:HALF, :],
                                     tsw[0:HALF, :])
                nc.gpsimd.tensor_add(dst[HALF:P, sl], tcos[HALF:P, :],
                                     tsw[HALF:P, :])

            def qk_group(b, ti, hh, w_w, dst):
                def emit():
                    sl = slice(ti * TQ, (ti + 1) * TQ)
                    xt = xts[(b, ti)]
                    hsl = slice(hh * D, (hh + 1) * D)
                    ps = ring_qk.tile([P, TQ], f32, tag="qk",
                                      name=f"qk{b}_{ti}_{hh}")
                    for ko in range(KO):
                        nc.tensor.matmul(ps[:], w_w(ko, hsl), xt[:, ko, :],
                                         start=(ko == 0), stop=(ko == KO - 1))
                    rope(ps, dst, sl, cs_sb[:, sl], sn_sb[:, sl])
                return (KO * TQ, emit)

            def v_group(b, ti, sub):
                def emit():
                    xt = xts[(b, ti)]
                    psv = ring_qk.tile([P, HPC * D], f32, tag="qk",
                                       name=f"v{b}_{ti}_{sub}")
                    for ko in range(KO):
                        nc.tensor.matmul(
                            psv[:], xt[:, ko, sub * P:(sub + 1) * P],
                            wv_sb[:, ko, :],
                            start=(ko == 0), stop=(ko == KO - 1))
                    tci = ti * (TQ // P) + sub
                    for hh in range(HPC):
                        nc.vector.tensor_copy(vt[b][hh][:, tci, :],
                                              psv[:, hh * D:(hh + 1) * D])
                return (KO * HPC * D, emit)

            def qkv_items(b):
                items = []
                wq_w = lambda ko, hsl: wq_g[ko // KG][:, ko % KG, hsl]
                wk_w = lambda ko, hsl: wk_sb[:, ko, hsl]
                for ti in range(NT):
                    for hh in range(HPC):
                        items.append(qk_group(b, ti, hh, wq_w, qT[b][hh]))
                    for hh in range(HPC):
                        items.append(qk_group(b, ti, hh, wk_w, kT[b][hh]))
                    for sub in range(TQ // P):
                        items.append(v_group(b, ti, sub))
                return items

            def attn_items(b):
                """Flat item list for batch b's attention: scores/exp chunks,
                AV+sum flushes lagging 2 slots, normalize lagging 3 more."""
                items = []
                norm_cd = []  # [countdown, (cost, fn)] pending normalizes

                def push(item):
                    items.append(item)
                    for e in norm_cd:
                        e[0] -= 1
                    while norm_cd and norm_cd[0][0] <= 0:
                        items.append(norm_cd.pop(0)[1])

                split = 0
                for hh in range(HPC):
                    if hh == 1:
                        split = len(items)
                    for ta in range(NTA):
                        nchunks = (ta + 1) * (TA // P)
                        st = {}

                        def chunk(s, hh=hh, ta=ta, st=st):
                            def emit():
                                diag = s >= ta * (TA // P)
                                t_lo = (s - ta * (TA // P)) * P if diag else 0
                                w = slice(t_lo, TA)
                                qsl = slice(ta * TA + t_lo, (ta + 1) * TA)
                                ps_sc = ring_sc.tile([P, TA], f32, tag="sc",
                                                     name=f"sc{b}{hh}{ta}_{s}")
                                nc.tensor.matmul(
                                    ps_sc[:, w],
                                    kT[b][hh][:, s * P:(s + 1) * P],
                                    qT[b][hh][:, qsl], start=True, stop=True)
                                pt = ptp.tile([P, TA], bf16, tag="pt",
                                              name=f"pt{b}{hh}{ta}_{s}")
                                nc.scalar.activation(
                                    pt[:, w], ps_sc[:, w],
                                    mybir.ActivationFunctionType.Exp,
                                    scale=SCALE)
                                if diag:  # mask the 128x128 triangle
                                    nc.vector.tensor_mul(
                                        pt[:, t_lo:t_lo + P],
                                        pt[:, t_lo:t_lo + P], mask_sb[:])
                                st[("pt", s)] = pt
                            return ((TA - (max(0, s - ta * (TA // P))) * P),
                                    emit)

                        def wsl(s, ta=ta):
                            diag = s >= ta * (TA // P)
                            t_lo = (s - ta * (TA // P)) * P if diag else 0
                            return slice(t_lo, TA)

                        def flush_av(s, hh=hh, ta=ta, st=st, nchunks=nchunks,
                                     wsl=wsl):
                            w = wsl(s)
                            if s == 0:
                                st["o"] = ring_acc.tile(
                                    [P, TA], f32, tag="o", name=f"o{b}{hh}{ta}")
                                st["sum"] = ring_sum.tile(
                                    [1, TA], f32, tag="sum",
                                    name=f"sum{b}{hh}{ta}")
                            nc.tensor.matmul(st["o"][:, w],
                                             vt[b][hh][:, s, :],
                                             st[("pt", s)][:, w],
                                             start=(s == 0),
                                             stop=(s == nchunks - 1))

                        def flush_sum(s, hh=hh, ta=ta, st=st, nchunks=nchunks,
                                      wsl=wsl):
                            w = wsl(s)
                            pt = st.pop(("pt", s))
                            nc.tensor.matmul(st["sum"][:, w], ones_col[:],
                                             pt[:, w], start=(s == 0),
                                             stop=(s == nchunks - 1))

                        def norm(hh=hh, ta=ta, st=st):
                            def emit():
                                recf = sp.tile([1, TA], f32, tag="recf",
                                               name="recf")
                                nc.vector.reciprocal_approx_fast(
                                    recf[:], st["sum"][:])
                                rec = sp.tile([1, TA], f32r, tag="rec",
                                              name="rec")
                                nc.vector.tensor_copy(rec[:], recf[:])
                                ps_bc = ring_sc.tile([P, TA], f32, tag="sc",
                                                     name=f"bc{b}{hh}{ta}")
                                nc.tensor.matmul(ps_bc[:], ones_row[:],
                                                 rec[:], start=True, stop=True)
                                bc_sb = sp.tile([P, TA], f32, tag="bc_sb",
                                                name="bc_sb")
                                nc.vector.tensor_copy(bc_sb[:], ps_bc[:])
                                # plain [d, t] oT write (contiguous DVE op);
                                # the out-projection shuffle happens in its
                                # strided stationary AP instead
                                nc.vector.tensor_mul(
                                    oT[b][hh][:, ta * TA:(ta + 1) * TA],
                                    st["o"][:], bc_sb[:])
                            return (TA, emit)

                        # quad weave: [c0..c3] [c4..c7] F(0..3) [c8..c11]
                        # F(4..7) ...  flush-quads emit AVx4 then sumx4 so
                        # consecutive matmuls share PSUM banks (fewer hops)
                        def flushquad(s0, fa=flush_av, fs=flush_sum, ta=ta):
                            def emit():
                                for s in range(s0, s0 + 4):
                                    fa(s)
                                for s in range(s0, s0 + 4):
                                    fs(s)
                            cost = sum(
                                2 * (TA - max(0, s - ta * (TA // P)) * P)
                                for s in range(s0, s0 + 4))
                            return (cost, emit)

                        for base in range(0, nchunks, 4):
                            for s in range(base, base + 4):
                                push(chunk(s))
                            if base >= 4:
                                push(flushquad(base - 4))
                        push(flushquad(nchunks - 4))
                        norm_cd.append([3, norm()])
                # leftover normalizes: paced into the next phase by the caller
                items.extend(e[1] for e in norm_cd)
                return items, split

            def outproj_items(b, half, heads):
                items = []
                for cpi in range(NTC):
                    for hh in heads:
                        def emit(cpi=cpi, hh=hh):
                            # contraction index k = (t%16)*128 + d; chunk j
                            # = t%16 needs oT columns {t = u*16+j} — strided
                            # stationary view of the plain [d, t] layout
                            ovw = oT[b][hh].rearrange("p (u j) -> p u j",
                                                      j=KO)
                            psy = ring_qk.tile([P, TC], f32, tag="qk",
                                               name=f"y{b}{hh}{cpi}")
                            for j in range(KO):
                                wt = wcs[(half, cpi, j // (KO // 2))]
                                nc.tensor.matmul(
                                    psy[:], ovw[:, :, j],
                                    wt[:, j % (KO // 2), :],
                                    start=(j == 0), stop=(j == KO - 1))
                            ysb = sp.tile([P, TC], f32, tag="ysb", name="ysb")
                            nc.vector.tensor_copy(ysb[:], psy[:])
                            nc.gpsimd.dma_start(
                                y.ap()[b, hh * D:(hh + 1) * D,
                                       cpi * TC:(cpi + 1) * TC], ysb[:])
                        items.append((KO * TC, emit))
                return items

            def merge(a_items, b_items, lead_a=0, bias=1.0):
                """Emit two independent item streams interleaved, pacing by
                cumulative estimated PE cost (each stream's internal order is
                preserved).  The first lead_a a-items are emitted before any
                b-item (emission order is dependency order: a b-item reading
                data produced by a deferred a-item must come after it)."""
                tot_a = sum(c for c, _ in a_items) or 1
                tot_b = sum(c for c, _ in b_items) or 1
                ca = cb = 0.0
                ia = ib = 0
                while ia < len(a_items) or ib < len(b_items):
                    take_a = ib >= len(b_items) or ia < lead_a or (
                        ia < len(a_items)
                        and ca / tot_a <= cb / (tot_b * bias))
                    if take_a:
                        c, fn = a_items[ia]; ia += 1; ca += c; fn()
                    else:
                        c, fn = b_items[ib]; ib += 1; cb += c; fn()

            def clump(items, k=2):
                # combine adjacent items so the merge inserts fewer
                # group<->chunk boundaries into the PE stream
                out = []
                for i in range(0, len(items), k):
                    grp = items[i:i + k]
                    fns = [f for _, f in grp]
                    out.append((sum(c for c, _ in grp),
                                lambda fns=fns: [f() for f in fns]))
                return out

            # ---------------- schedule ----------------
            for _, fn in qkv_items(0):              # P1
                fn()
            merge(attn_items(0)[0], clump(qkv_items(1)))   # P2
            # P3: attention(b1) with b0's out-projection as PE cover,
            # front-loaded (bias) so P4's wout copies land before P4 starts
            merge(attn_items(1)[0], outproj_items(0, 0, (0, 1)), bias=1.2)
            for _, fn in outproj_items(1, 1, (0, 1)):       # P4
                fn()

    nc.compile()
    return nc


_NC = None


def _get_nc():
    global _NC
    if _NC is None:
        _NC = _build()
    return _NC


def _host_tables():
    pos = np.arange(T, dtype=np.float32)[:, None]
    div = np.exp(np.arange(0, 2 * HALF, 2, dtype=np.float32)
                 * np.float32(-math.log(ROPE_BASE) / (2 * HALF)))
    ang = pos * div[None, :]
    cosv = np.cos(ang).astype(np.float32)   # [T, HALF]
    sinv = np.sin(ang).astype(np.float32)
    cosT = np.ascontiguousarray(cosv.T)     # [HALF, T]
    sinT = np.ascontiguousarray(sinv.T)
    cs2 = np.ascontiguousarray(
        np.concatenate([cosT, cosT], axis=0)).astype(npbf16)  # [P, T]
    sn1 = sinT.astype(npbf16)
    # triangle mask M[s, w] = 1 iff s <= w
    ww = np.arange(P)[None, :]
    ss = np.arange(P)[:, None]
    maskM = (ss <= ww).astype(npbf16)
    return cs2, sn1, maskM


def _make_in_maps(x, Wqkv, Wout):
    x = np.asarray(x, dtype=np.float32)
    Wqkv = np.asarray(Wqkv, dtype=np.float32)
    Wout = np.asarray(Wout, dtype=np.float32)
    assert x.shape == (B, T, C) and Wqkv.shape == (C, 3 * C) \
        and Wout.shape == (C, C)

    cs2, sn1, maskM = _host_tables()
    # xTt[b, ti, p, ko, u] = x[b, ti*TQ+u, ko*128+p]
    xTt = np.ascontiguousarray(
        x.reshape(B, NT, TQ, KO, P).transpose(0, 1, 4, 3, 2).astype(npbf16))
    # wout4[cpi, p, j, u] = Wout[j*128+p, cpi*TC+u]
    wout4 = np.ascontiguousarray(
        Wout.reshape(KO, P, NTC, TC).transpose(2, 1, 0, 3).astype(npbf16))

    in_maps = []
    for core in range(NCORES):
        h0 = core * HPC
        cols = slice(h0 * D, (h0 + HPC) * D)
        ws = []
        for part in range(3):
            w = Wqkv[:, part * C:(part + 1) * C][:, cols]  # [C, HPC*D]
            ws.append(np.ascontiguousarray(
                w.reshape(KO, P, HPC * D).transpose(1, 0, 2).astype(npbf16)))
        in_maps.append({
            "xTt": xTt,
            "wq": ws[0], "wk": ws[1], "wv": ws[2],
            "wout": wout4,
            "cs2": cs2, "sn1": sn1, "maskM": maskM,
        })
    return in_maps


def _run(x, Wqkv, Wout, trace=False):
    nc = _get_nc()
    in_maps = _make_in_maps(x, Wqkv, Wout)
    res = run_bass_kernel_spmd(nc, in_maps, core_ids=list(range(NCORES)),
                               trace=trace)
    out = np.empty((B, T, C), dtype=np.float32)
    for core in range(NCORES):
        out[:, core * HPC * D:(core + 1) * HPC * D, :] = \
            res.results[core]["y"]
    return out, res


def kernel(x, Wqkv, Wout):
    out, _ = _run(x, Wqkv, Wout)
    return out


# revision 67
# speedup vs baseline: 1.0133x; 1.0133x over previous
"""Trainium2 Bass kernel for nn_MultiHeadAttention_63015760167496.

Computation (see reference): qkv = x @ Wqkv; RoPE on q,k; causal softmax
attention per head; out = einsum('bhts,bshd->bhtd', probs, v);
out.reshape(B,T,C) @ Wout  -- NOTE the reshape is a *head-major* flatten of
[B,H,T,D] into [B,T,C], so final-output row r = h*128 + t//16 depends only on
head h.  Sharding: head-parallel over 8 cores (2 heads/core); every core
computes its two heads end-to-end and produces final-output rows
[256*i, 256*i+256).  Host concatenates -- no collectives.

v3: all matmul payloads bf16 (host-cast); 512-wide tiles; the whole kernel is
one continuous PE instruction stream with cross-phase software pipelining:
  P1 = qkv(b0)
  P2 = attention(b0) interleaved with qkv(b1)
  P3 = attention(b1) interleaved with out-projection of b0's heads
  P4 = out-projection of b1's heads
The interleave keeps the tensor engine busy back-to-back (sustaining the
2.4 GHz p-state) and hides the scalar-engine exp + the softmax-normalize
dependency chains behind independent GEMM groups.  Attention is computed in
S^T layout ([s,t]): denominator via a ones-vector matmul, normalization via a
K=1 broadcast matmul; AV/sum flushes lag their scores chunk by 2 slots and
each tile's normalize lags 3 more items.  PSUM is four top-level rings shared
by all phases.
"""

import math
import sys

for _p in ("/opt/trn_rl_repo", "/root/.axon_site/_ro/trn_rl_repo"):
    if _p not in sys.path:
        sys.path.insert(0, _p)

import numpy as np
import ml_dtypes

import concourse.bass as bass
import concourse.mybir as mybir
import concourse.tile as tile
from concourse import bacc
from concourse.bass_utils import run_bass_kernel_spmd

B, T, C = 2, 2048, 2048
H = 16            # heads total
D = C // H        # 128 head dim
HALF = D // 2     # 64
P = 128
KO = C // P       # 16 contraction chunks
KG = 4            # ko chunks per startup DMA group
NG = KO // KG     # 4 startup groups
NCORES = 8
HPC = H // NCORES  # 2 heads per core
TQ = 512          # t-tile for qkv projection
NT = T // TQ      # 4
TA = 512          # t-tile for attention
NTA = T // TA     # 4
NSC = T // P      # 16 s-chunks
TC = 512          # col-tile for out projection
NTC = C // TC     # 4
ROPE_BASE = 10000.0
SCALE = 1.0 / math.sqrt(D)

f32 = mybir.dt.float32
f32r = mybir.dt.float32r
bf16 = mybir.dt.bfloat16
npbf16 = ml_dtypes.bfloat16


def _build():
    nc = bacc.Bacc("TRN2", target_bir_lowering=False, debug=False,
                   num_devices=NCORES)

    # host-pre-tiled x^T: xTt[b, ti, p, ko, u] = x[b, ti*TQ+u, ko*128+p]
    xTt = nc.dram_tensor("xTt", [B, NT, P, KO, TQ], bf16, kind="ExternalInput")
    # host-pre-chunked weights: w[p, ko, m] = W[ko*128+p, m]
    wq = nc.dram_tensor("wq", [P, KO, HPC * D], bf16, kind="ExternalInput")
    wk = nc.dram_tensor("wk", [P, KO, HPC * D], bf16, kind="ExternalInput")
    wv = nc.dram_tensor("wv", [P, KO, HPC * D], bf16, kind="ExternalInput")
    # wout4[cpi, p, j, u] = Wout[j*128+p, cpi*TC+u]
    wout = nc.dram_tensor("wout", [NTC, P, KO, TC], bf16, kind="ExternalInput")
    cs2 = nc.dram_tensor("cs2", [P, T], bf16, kind="ExternalInput")  # [cos;cos]
    sn1 = nc.dram_tensor("sn1", [HALF, T], bf16, kind="ExternalInput")  # sin
    maskM = nc.dram_tensor("maskM", [P, P], bf16, kind="ExternalInput")
    y = nc.dram_tensor("y", [B, HPC * D, C], f32, kind="ExternalOutput")

    with tile.TileContext(nc) as tc:
        with tc.tile_pool(name="const", bufs=1) as cp_, \
             tc.tile_pool(name="qkv", bufs=1) as qp, \
             tc.tile_pool(name="ot", bufs=1) as op_, \
             tc.tile_pool(name="xt", bufs=2) as xp, \
             tc.tile_pool(name="wcp", bufs=2) as wp, \
             tc.tile_pool(name="rope", bufs=2) as rp, \
             tc.tile_pool(name="pt", bufs=8) as ptp, \
             tc.tile_pool(name="small", bufs=2) as sp, \
             tc.tile_pool(name="psqk", bufs=3, space="PSUM") as ring_qk, \
             tc.tile_pool(name="pssc", bufs=2, space="PSUM") as ring_sc, \
             tc.tile_pool(name="psacc", bufs=2, space="PSUM") as ring_acc, \
             tc.tile_pool(name="pssum", bufs=1, space="PSUM") as ring_sum:

            wq_g = [cp_.tile([P, KG, HPC * D], bf16, tag=f"wqg{g}",
                             name=f"wqg{g}") for g in range(NG)]
            wk_sb = cp_.tile([P, KO, HPC * D], bf16, tag="wk")
            wv_sb = cp_.tile([P, KO, HPC * D], bf16, tag="wv")
            cs_sb = cp_.tile([P, T], bf16, tag="cs")
            sn_sb = cp_.tile([HALF, T], bf16, tag="sn")
            mask_sb = cp_.tile([P, P], bf16, tag="mask")

            ones_f32 = cp_.tile([P, 1], f32, tag="ones_f32")
            nc.vector.memset(ones_f32[:], 1.0)
            ones_col = cp_.tile([P, 1], bf16, tag="ones_col")
            nc.vector.tensor_copy(ones_col[:], ones_f32[:])
            onesr_f32 = cp_.tile([1, P], f32, tag="onesr_f32")
            nc.vector.memset(onesr_f32[:], 1.0)
            ones_row = cp_.tile([1, P], f32r, tag="ones_row")
            nc.vector.tensor_copy(ones_row[:], onesr_f32[:])

            # startup DMAs: issue from four parallel engine queues (the sync
            # sequencer takes ~650ns per issue — serializing all of these
            # on one queue costs ~6us before the last transfer even starts).
            # wq on sync, xt00 on vector, wk/wv on scalar, cos/sin on gpsimd.
            xt00 = xp.tile([P, KO, TQ], bf16, tag="xt", name="xt00")
            for g in range(NG):
                ksl = slice(g * KG, (g + 1) * KG)
                nc.sync.dma_start(wq_g[g][:], wq.ap()[:, ksl])
                nc.sync.dma_start(xt00[:, ksl, :], xTt.ap()[0, 0, :, ksl])
            nc.scalar.dma_start(wk_sb[:, 0:KO // 2], wk.ap()[:, 0:KO // 2])
            nc.scalar.dma_start(wk_sb[:, KO // 2:KO], wk.ap()[:, KO // 2:KO])
            nc.gpsimd.dma_start(cs_sb[:], cs2.ap())
            nc.gpsimd.dma_start(sn_sb[:], sn1.ap())
            nc.scalar.dma_start(wv_sb[:, 0:KO // 2], wv.ap()[:, 0:KO // 2])
            nc.scalar.dma_start(wv_sb[:, KO // 2:KO], wv.ap()[:, KO // 2:KO])
            nc.sync.dma_start(mask_sb[:], maskM.ap())

            # remaining x tiles; WAR on the pool ring paces the sync queue
            xts = {(0, 0): xt00}
            for b in range(B):
                for ti in range(NT):
                    if b == 0 and ti == 0:
                        continue
                    xt = xp.tile([P, KO, TQ], bf16, tag="xt",
                                 name=f"xt{b}{ti}")
                    nc.sync.dma_start(xt[:], xTt.ap()[b, ti])
                    xts[(b, ti)] = xt
            # wout slices: one copy per consuming half-pass (b0 pass in P3,
            # b1 pass in P4) so the ring-2 WAR chain stays acyclic.
            # two out-projection passes (b0 both heads, then b1), each with
            # its own wout copy so the ring-2 WAR chain stays acyclic
            # half-size tiles in a ring of 4 (same SBUF footprint as two
            # full tiles): each WAR-gated DMA is 1MB, landing sooner
            wcs = {}
            for half in range(2):
                for cpi in range(NTC):
                    for part in range(2):
                        w = wp.tile([P, KO // 2, TC], bf16, tag="wc", bufs=5,
                                    name=f"wc{half}_{cpi}_{part}")
                        for q4 in range(2):
                            jsl = slice(q4 * KO // 4, (q4 + 1) * KO // 4)
                            gsl = slice(part * KO // 2 + q4 * KO // 4,
                                        part * KO // 2 + (q4 + 1) * KO // 4)
                            nc.sync.dma_start(w[:, jsl, :],
                                              wout.ap()[cpi, :, gsl])
                        wcs[(half, cpi, part)] = w

            # persistent tensors per (b, local head)
            oT = [[op_.tile([P, T], bf16, tag=f"oT{b}{hh}", name=f"oT{b}{hh}")
                   for hh in range(HPC)] for b in range(B)]
            qT = [[qp.tile([P, T], bf16, tag=f"qT{b}{hh}", name=f"qT{b}{hh}")
                   for hh in range(HPC)] for b in range(B)]
            kT = [[qp.tile([P, T], bf16, tag=f"kT{b}{hh}", name=f"kT{b}{hh}")
                   for hh in range(HPC)] for b in range(B)]
            vt = [[qp.tile([P, NSC, D], bf16, tag=f"v{b}{hh}",
                           name=f"v{b}{hh}") for hh in range(HPC)]
                  for b in range(B)]

            # ---------------- emitters ----------------
            def rope(ps, dst, sl, cs, sn):
                # tcos = ps * [cos;cos]; tsw pre-swaps halves (tsw[0:64] =
                # q2*sin, tsw[64:128] = q1*sin) so reads align on base
                # partitions; muls on DVE (psum-capable), sub/add on gpsimd.
                tcos = rp.tile([P, TQ], f32, tag="tcos", name="tcos")
                tsw = rp.tile([P, TQ], f32, tag="tsw", name="tsw")
                nc.vector.tensor_mul(tcos[:], ps[:], cs)
                nc.vector.tensor_mul(tsw[0:HALF, :], ps[HALF:P, :], sn)
                nc.vector.tensor_mul(tsw[HALF:P, :], ps[0:HALF, :], sn)
                nc.gpsimd.tensor_sub(dst[0:HALF, sl], tcos[0:HALF, :],
                                     tsw[0:HALF, :])
                nc.gpsimd.tensor_add(dst[HALF:P, sl], tcos[HALF:P, :],
                                     tsw[HALF:P, :])

            def qk_group(b, ti, hh, w_w, dst):
                def emit():
                    sl = slice(ti * TQ, (ti + 1) * TQ)
                    xt = xts[(b, ti)]
                    hsl = slice(hh * D, (hh + 1) * D)
                    ps = ring_qk.tile([P, TQ], f32, tag="qk",
                                      name=f"qk{b}_{ti}_{hh}")
                    for ko in range(KO):
                        nc.tensor.matmul(ps[:], w_w(ko, hsl), xt[:, ko, :],
                                         start=(ko == 0), stop=(ko == KO - 1))
                    rope(ps, dst, sl, cs_sb[:, sl], sn_sb[:, sl])
                return (KO * TQ, emit)

            def v_group(b, ti, sub):
                def emit():
                    xt = xts[(b, ti)]
                    psv = ring_qk.tile([P, HPC * D], f32, tag="qk",
                                       name=f"v{b}_{ti}_{sub}")
                    for ko in range(KO):
                        nc.tensor.matmul(
                            psv[:], xt[:, ko, sub * P:(sub + 1) * P],
                            wv_sb[:, ko, :],
                            start=(ko == 0), stop=(ko == KO - 1))
                    tci = ti * (TQ // P) + sub
                    for hh in range(HPC):
                        nc.vector.tensor_copy(vt[b][hh][:, tci, :],
                                              psv[:, hh * D:(hh + 1) * D])
                return (KO * HPC * D, emit)

            def qkv_items(b):
                items = []
                wq_w = lambda ko, hsl: wq_g[ko // KG][:, ko % KG, hsl]
                wk_w = lambda ko, hsl: wk_sb[:, ko, hsl]
                for ti in range(NT):
                    for hh in range(HPC):
                        items.append(qk_group(b, ti, hh, wq_w, qT[b][hh]))
                    for hh in range(HPC):
                        items.append(qk_group(b, ti, hh, wk_w, kT[b][hh]))
                    for sub in range(TQ // P):
                        items.append(v_group(b, ti, sub))
                return items

            def attn_items(b):
                """Flat item list for batch b's attention: scores/exp chunks,
                AV+sum flushes lagging 2 slots, normalize lagging 3 more."""
                items = []
                norm_cd = []  # [countdown, (cost, fn)] pending normalizes

                def push(item):
                    items.append(item)
                    for e in norm_cd:
                        e[0] -= 1
                    while norm_cd and norm_cd[0][0] <= 0:
                        items.append(norm_cd.pop(0)[1])

                split = 0
                for hh in range(HPC):
                    if hh == 1:
                        split = len(items)
                    for ta in range(NTA):
                        nchunks = (ta + 1) * (TA // P)
                        st = {}

                        def chunk(s, hh=hh, ta=ta, st=st):
                            def emit():
                                diag = s >= ta * (TA // P)
                                t_lo = (s - ta * (TA // P)) * P if diag else 0
                                w = slice(t_lo, TA)
                                qsl = slice(ta * TA + t_lo, (ta + 1) * TA)
                                ps_sc = ring_sc.tile([P, TA], f32, tag="sc",
                                                     name=f"sc{b}{hh}{ta}_{s}")
                                nc.tensor.matmul(
                                    ps_sc[:, w],
                                    kT[b][hh][:, s * P:(s + 1) * P],
                                    qT[b][hh][:, qsl], start=True, stop=True)
                                pt = ptp.tile([P, TA], bf16, tag="pt",
                                              name=f"pt{b}{hh}{ta}_{s}")
                                nc.scalar.activation(
                                    pt[:, w], ps_sc[:, w],
                                    mybir.ActivationFunctionType.Exp,
                                    scale=SCALE)
                                if diag:  # mask the 128x128 triangle
                                    nc.vector.tensor_mul(
                                        pt[:, t_lo:t_lo + P],
                                        pt[:, t_lo:t_lo + P], mask_sb[:])
                                st[("pt", s)] = pt
                            return ((TA - (max(0, s - ta * (TA // P))) * P),
                                    emit)

                        def wsl(s, ta=ta):
                            diag = s >= ta * (TA // P)
                            t_lo = (s - ta * (TA // P)) * P if diag else 0
                            return slice(t_lo, TA)

                        def flush_av(s, hh=hh, ta=ta, st=st, nchunks=nchunks,
                                     wsl=wsl):
                            w = wsl(s)
                            if s == 0:
                                st["o"] = ring_acc.tile(
                                    [P, TA], f32, tag="o", name=f"o{b}{hh}{ta}")
                                st["sum"] = ring_sum.tile(
                                    [1, TA], f32, tag="sum",
                                    name=f"sum{b}{hh}{ta}")
                            nc.tensor.matmul(st["o"][:, w],
                                             vt[b][hh][:, s, :],
                                             st[("pt", s)][:, w],
                                             start=(s == 0),
                                             stop=(s == nchunks - 1))

                        def flush_sum(s, hh=hh, ta=ta, st=st, nchunks=nchunks,
                                      wsl=wsl):
                            w = wsl(s)
                            pt = st.pop(("pt", s))
                            nc.tensor.matmul(st["sum"][:, w], ones_col[:],
                                             pt[:, w], start=(s == 0),
                                             stop=(s == nchunks - 1))

                        def norm(hh=hh, ta=ta, st=st):
                            def emit():
                                recf = sp.tile([1, TA], f32, tag="recf",
                                               name="recf")
                                nc.vector.reciprocal_approx_fast(
                                    recf[:], st["sum"][:])
                                rec = sp.tile([1, TA], f32r, tag="rec",
                                              name="rec")
                                nc.vector.tensor_copy(rec[:], recf[:])
                                ps_bc = ring_sc.tile([P, TA], f32, tag="sc",
                                                     name=f"bc{b}{hh}{ta}")
                                nc.tensor.matmul(ps_bc[:], ones_row[:],
                                                 rec[:], start=True, stop=True)
                                bc_sb = sp.tile([P, TA], f32, tag="bc_sb",
                                                name="bc_sb")
                                nc.vector.tensor_copy(bc_sb[:], ps_bc[:])
                                # plain [d, t] oT write (contiguous DVE op);
                                # the out-projection shuffle happens in its
                                # strided stationary AP instead
                                nc.vector.tensor_mul(
                                    oT[b][hh][:, ta * TA:(ta + 1) * TA],
                                    st["o"][:], bc_sb[:])
                            return (TA, emit)

                        # quad weave: [c0..c3] [c4..c7] F(0..3) [c8..c11]
                        # F(4..7) ...  flush-quads emit AVx4 then sumx4 so
                        # consecutive matmuls share PSUM banks (fewer hops)
                        def flushquad(s0, fa=flush_av, fs=flush_sum, ta=ta):
                            def emit():
                                for s in range(s0, s0 + 4):
                                    fa(s)
                                for s in range(s0, s0 + 4):
                                    fs(s)
                            cost = sum(
                                2 * (TA - max(0, s - ta * (TA // P)) * P)
                                for s in range(s0, s0 + 4))
                            return (cost, emit)

                        for base in range(0, nchunks, 4):
                            for s in range(base, base + 4):
                                push(chunk(s))
                            if base >= 4:
                                push(flushquad(base - 4))
                        push(flushquad(nchunks - 4))
                        norm_cd.append([3, norm()])
                # leftover normalizes: paced into the next phase by the caller
                items.extend(e[1] for e in norm_cd)
                return items, split

            def outproj_items(b, half, heads):
                items = []
                for cpi in range(NTC):
                    for hh in heads:
                        def emit(cpi=cpi, hh=hh):
                            # contraction index k = (t%16)*128 + d; chunk j
                            # = t%16 needs oT columns {t = u*16+j} — strided
                            # stationary view of the plain [d, t] layout
                            ovw = oT[b][hh].rearrange("p (u j) -> p u j",
                                                      j=KO)
                            psy = ring_qk.tile([P, TC], f32, tag="qk",
                                               name=f"y{b}{hh}{cpi}")
                            for j in range(KO):
                                wt = wcs[(half, cpi, j // (KO // 2))]
                                nc.tensor.matmul(
                                    psy[:], ovw[:, :, j],
                                    wt[:, j % (KO // 2), :],
                                    start=(j == 0), stop=(j == KO - 1))
                            ysb = sp.tile([P, TC], f32, tag="ysb", name="ysb")
                            nc.vector.tensor_copy(ysb[:], psy[:])
                            nc.gpsimd.dma_start(
                                y.ap()[b, hh * D:(hh + 1) * D,
                                       cpi * TC:(cpi + 1) * TC], ysb[:])
                        items.append((KO * TC, emit))
                return items

            def merge(a_items, b_items, lead_a=0, bias=1.0):
                """Emit two independent item streams interleaved, pacing by
                cumulative estimated PE cost (each stream's internal order is
                preserved).  The first lead_a a-items are emitted before any
                b-item (emission order is dependency order: a b-item reading
                data produced by a deferred a-item must come after it)."""
                tot_a = sum(c for c, _ in a_items) or 1
                tot_b = sum(c for c, _ in b_items) or 1
                ca = cb = 0.0
                ia = ib = 0
                while ia < len(a_items) or ib < len(b_items):
                    take_a = ib >= len(b_items) or ia < lead_a or (
                        ia < len(a_items)
                        and ca / tot_a <= cb / (tot_b * bias))
                    if take_a:
                        c, fn = a_items[ia]; ia += 1; ca += c; fn()
                    else:
                        c, fn = b_items[ib]; ib += 1; cb += c; fn()

            def clump(items, k=2):
                # combine adjacent items so the merge inserts fewer
                # group<->chunk boundaries into the PE stream
                out = []
                for i in range(0, len(items), k):
                    grp = items[i:i + k]
                    fns = [f for _, f in grp]
                    out.append((sum(c for c, _ in grp),
                                lambda fns=fns: [f() for f in fns]))
                return out

            # ---------------- schedule ----------------
            for _, fn in qkv_items(0):              # P1
                fn()
            merge(attn_items(0)[0], clump(qkv_items(1)))   # P2
            # P3: attention(b1) with b0's out-projection as PE cover,
            # front-loaded (bias) so P4's wout copies land before P4 starts
            merge(attn_items(1)[0], outproj_items(0, 0, (0, 1)), bias=1.2)
            for _, fn in outproj_items(1, 1, (0, 1)):       # P4
                fn()

    nc.compile()
    return nc


_NC = None


def _get_nc():
    global _NC
    if _NC is None:
        _NC = _build()
    return _NC


def _host_tables():
    pos = np.arange(T, dtype=np.float32)[:, None]
    div = np.exp(np.arange(0, 2 * HALF, 2, dtype=np.float32)
                 * np.float32(-math.log(ROPE_BASE) / (2 * HALF)))
    ang = pos * div[None, :]
    cosv = np.cos(ang).astype(np.float32)   # [T, HALF]
    sinv = np.sin(ang).astype(np.float32)
    cosT = np.ascontiguousarray(cosv.T)     # [HALF, T]
    sinT = np.ascontiguousarray(sinv.T)
    cs2 = np.ascontiguousarray(
        np.concatenate([cosT, cosT], axis=0)).astype(npbf16)  # [P, T]
    sn1 = sinT.astype(npbf16)
    # triangle mask M[s, w] = 1 iff s <= w
    ww = np.arange(P)[None, :]
    ss = np.arange(P)[:, None]
    maskM = (ss <= ww).astype(npbf16)
    return cs2, sn1, maskM


def _make_in_maps(x, Wqkv, Wout):
    x = np.asarray(x, dtype=np.float32)
    Wqkv = np.asarray(Wqkv, dtype=np.float32)
    Wout = np.asarray(Wout, dtype=np.float32)
    assert x.shape == (B, T, C) and Wqkv.shape == (C, 3 * C) \
        and Wout.shape == (C, C)

    cs2, sn1, maskM = _host_tables()
    # xTt[b, ti, p, ko, u] = x[b, ti*TQ+u, ko*128+p]
    xTt = np.ascontiguousarray(
        x.reshape(B, NT, TQ, KO, P).transpose(0, 1, 4, 3, 2).astype(npbf16))
    # wout4[cpi, p, j, u] = Wout[j*128+p, cpi*TC+u]
    wout4 = np.ascontiguousarray(
        Wout.reshape(KO, P, NTC, TC).transpose(2, 1, 0, 3).astype(npbf16))

    in_maps = []
    for core in range(NCORES):
        h0 = core * HPC
        cols = slice(h0 * D, (h0 + HPC) * D)
        ws = []
        for part in range(3):
            w = Wqkv[:, part * C:(part + 1) * C][:, cols]  # [C, HPC*D]
            ws.append(np.ascontiguousarray(
                w.reshape(KO, P, HPC * D).transpose(1, 0, 2).astype(npbf16)))
        in_maps.append({
            "xTt": xTt,
            "wq": ws[0], "wk": ws[1], "wv": ws[2],
            "wout": wout4,
            "cs2": cs2, "sn1": sn1, "maskM": maskM,
        })
    return in_maps


def _run(x, Wqkv, Wout, trace=False):
    nc = _get_nc()
    in_maps = _make_in_maps(x, Wqkv, Wout)
    res = run_bass_kernel_spmd(nc, in_maps, core_ids=list(range(NCORES)),
                               trace=trace)
    out = np.empty((B, T, C), dtype=np.float32)
    for core in range(NCORES):
        out[:, core * HPC * D:(core + 1) * HPC * D, :] = \
            res.results[core]["y"]
    return out, res


def kernel(x, Wqkv, Wout):
    out, _ = _run(x, Wqkv, Wout)
    return out


# revision 68
# speedup vs baseline: 1.0279x; 1.0144x over previous
"""Trainium2 Bass kernel for nn_MultiHeadAttention_63015760167496.

Computation (see reference): qkv = x @ Wqkv; RoPE on q,k; causal softmax
attention per head; out = einsum('bhts,bshd->bhtd', probs, v);
out.reshape(B,T,C) @ Wout  -- NOTE the reshape is a *head-major* flatten of
[B,H,T,D] into [B,T,C], so final-output row r = h*128 + t//16 depends only on
head h.  Sharding: head-parallel over 8 cores (2 heads/core); every core
computes its two heads end-to-end and produces final-output rows
[256*i, 256*i+256).  Host concatenates -- no collectives.

v3: all matmul payloads bf16 (host-cast); 512-wide tiles; the whole kernel is
one continuous PE instruction stream with cross-phase software pipelining:
  P1 = qkv(b0)
  P2 = attention(b0) interleaved with qkv(b1)
  P3 = attention(b1) interleaved with out-projection of b0's heads
  P4 = out-projection of b1's heads
The interleave keeps the tensor engine busy back-to-back (sustaining the
2.4 GHz p-state) and hides the scalar-engine exp + the softmax-normalize
dependency chains behind independent GEMM groups.  Attention is computed in
S^T layout ([s,t]): denominator via a ones-vector matmul, normalization via a
K=1 broadcast matmul; AV/sum flushes lag their scores chunk by 2 slots and
each tile's normalize lags 3 more items.  PSUM is four top-level rings shared
by all phases.
"""

import math
import sys

for _p in ("/opt/trn_rl_repo", "/root/.axon_site/_ro/trn_rl_repo"):
    if _p not in sys.path:
        sys.path.insert(0, _p)

import numpy as np
import ml_dtypes

import concourse.bass as bass
import concourse.mybir as mybir
import concourse.tile as tile
from concourse import bacc
from concourse.bass_utils import run_bass_kernel_spmd

B, T, C = 2, 2048, 2048
H = 16            # heads total
D = C // H        # 128 head dim
HALF = D // 2     # 64
P = 128
KO = C // P       # 16 contraction chunks
KG = 4            # ko chunks per startup DMA group
NG = KO // KG     # 4 startup groups
NCORES = 8
HPC = H // NCORES  # 2 heads per core
TQ = 512          # t-tile for qkv projection
NT = T // TQ      # 4
TA = 512          # t-tile for attention
NTA = T // TA     # 4
NSC = T // P      # 16 s-chunks
TC = 512          # col-tile for out projection
NTC = C // TC     # 4
ROPE_BASE = 10000.0
SCALE = 1.0 / math.sqrt(D)

f32 = mybir.dt.float32
f32r = mybir.dt.float32r
bf16 = mybir.dt.bfloat16
npbf16 = ml_dtypes.bfloat16


def _build():
    nc = bacc.Bacc("TRN2", target_bir_lowering=False, debug=False,
                   num_devices=NCORES)

    # host-pre-tiled x^T: xTt[b, ti, p, ko, u] = x[b, ti*TQ+u, ko*128+p]
    xTt = nc.dram_tensor("xTt", [B, NT, P, KO, TQ], bf16, kind="ExternalInput")
    # host-pre-chunked weights: w[p, ko, m] = W[ko*128+p, m]
    wq = nc.dram_tensor("wq", [P, KO, HPC * D], bf16, kind="ExternalInput")
    wk = nc.dram_tensor("wk", [P, KO, HPC * D], bf16, kind="ExternalInput")
    wv = nc.dram_tensor("wv", [P, KO, HPC * D], bf16, kind="ExternalInput")
    # wout4[cpi, p, j, u] = Wout[j*128+p, cpi*TC+u]
    wout = nc.dram_tensor("wout", [NTC, P, KO, TC], bf16, kind="ExternalInput")
    cs2 = nc.dram_tensor("cs2", [P, T], bf16, kind="ExternalInput")  # [cos;cos]
    sn1 = nc.dram_tensor("sn1", [HALF, T], bf16, kind="ExternalInput")  # sin
    maskM = nc.dram_tensor("maskM", [P, P], bf16, kind="ExternalInput")
    y = nc.dram_tensor("y", [B, HPC * D, C], f32, kind="ExternalOutput")

    with tile.TileContext(nc) as tc:
        with tc.tile_pool(name="const", bufs=1) as cp_, \
             tc.tile_pool(name="qkv", bufs=1) as qp, \
             tc.tile_pool(name="ot", bufs=1) as op_, \
             tc.tile_pool(name="xt", bufs=2) as xp, \
             tc.tile_pool(name="wcp", bufs=2) as wp, \
             tc.tile_pool(name="rope", bufs=2) as rp, \
             tc.tile_pool(name="pt", bufs=8) as ptp, \
             tc.tile_pool(name="small", bufs=2) as sp, \
             tc.tile_pool(name="psqk", bufs=3, space="PSUM") as ring_qk, \
             tc.tile_pool(name="pssc", bufs=2, space="PSUM") as ring_sc, \
             tc.tile_pool(name="psacc", bufs=2, space="PSUM") as ring_acc, \
             tc.tile_pool(name="pssum", bufs=1, space="PSUM") as ring_sum:

            wq_g = [cp_.tile([P, KG, HPC * D], bf16, tag=f"wqg{g}",
                             name=f"wqg{g}") for g in range(NG)]
            wk_sb = cp_.tile([P, KO, HPC * D], bf16, tag="wk")
            wv_sb = cp_.tile([P, KO, HPC * D], bf16, tag="wv")
            cs_sb = cp_.tile([P, T], bf16, tag="cs")
            sn_sb = cp_.tile([HALF, T], bf16, tag="sn")
            mask_sb = cp_.tile([P, P], bf16, tag="mask")

            ones_f32 = cp_.tile([P, 1], f32, tag="ones_f32")
            nc.vector.memset(ones_f32[:], 1.0)
            ones_col = cp_.tile([P, 1], bf16, tag="ones_col")
            nc.vector.tensor_copy(ones_col[:], ones_f32[:])
            onesr_f32 = cp_.tile([1, P], f32, tag="onesr_f32")
            nc.vector.memset(onesr_f32[:], 1.0)
            ones_row = cp_.tile([1, P], f32r, tag="ones_row")
            nc.vector.tensor_copy(ones_row[:], onesr_f32[:])

            # startup DMAs: issue from four parallel engine queues (the sync
            # sequencer takes ~650ns per issue — serializing all of these
            # on one queue costs ~6us before the last transfer even starts).
            # wq on sync, xt00 on vector, wk/wv on scalar, cos/sin on gpsimd.
            xt00 = xp.tile([P, KO, TQ], bf16, tag="xt", name="xt00")
            # single sync queue: in-order issue doubles as bandwidth
            # priority for the startup-critical transfers (parallel engine
            # queues dilute the first tiles' share of DMA bandwidth)
            for g in range(NG):
                ksl = slice(g * KG, (g + 1) * KG)
                nc.sync.dma_start(wq_g[g][:], wq.ap()[:, ksl])
                nc.sync.dma_start(xt00[:, ksl, :], xTt.ap()[0, 0, :, ksl])
                if g == 1:
                    nc.sync.dma_start(wk_sb[:, 0:KO // 2],
                                      wk.ap()[:, 0:KO // 2])
            nc.sync.dma_start(wk_sb[:, KO // 2:KO], wk.ap()[:, KO // 2:KO])
            nc.sync.dma_start(cs_sb[:], cs2.ap())
            nc.sync.dma_start(sn_sb[:], sn1.ap())
            nc.sync.dma_start(wv_sb[:, 0:KO // 2], wv.ap()[:, 0:KO // 2])
            nc.sync.dma_start(wv_sb[:, KO // 2:KO], wv.ap()[:, KO // 2:KO])
            nc.sync.dma_start(mask_sb[:], maskM.ap())

            # remaining x tiles; WAR on the pool ring paces the sync queue
            xts = {(0, 0): xt00}
            for b in range(B):
                for ti in range(NT):
                    if b == 0 and ti == 0:
                        continue
                    xt = xp.tile([P, KO, TQ], bf16, tag="xt",
                                 name=f"xt{b}{ti}")
                    nc.sync.dma_start(xt[:], xTt.ap()[b, ti])
                    xts[(b, ti)] = xt
            # wout slices: one copy per consuming half-pass (b0 pass in P3,
            # b1 pass in P4) so the ring-2 WAR chain stays acyclic.
            # two out-projection passes (b0 both heads, then b1), each with
            # its own wout copy so the ring-2 WAR chain stays acyclic
            # half-size tiles in a ring of 4 (same SBUF footprint as two
            # full tiles): each WAR-gated DMA is 1MB, landing sooner
            wcs = {}
            for half in range(2):
                for cpi in range(NTC):
                    for part in range(2):
                        w = wp.tile([P, KO // 2, TC], bf16, tag="wc", bufs=5,
                                    name=f"wc{half}_{cpi}_{part}")
                        for q4 in range(2):
                            jsl = slice(q4 * KO // 4, (q4 + 1) * KO // 4)
                            gsl = slice(part * KO // 2 + q4 * KO // 4,
                                        part * KO // 2 + (q4 + 1) * KO // 4)
                            nc.sync.dma_start(w[:, jsl, :],
                                              wout.ap()[cpi, :, gsl])
                        wcs[(half, cpi, part)] = w

            # persistent tensors per (b, local head)
            oT = [[op_.tile([P, T], bf16, tag=f"oT{b}{hh}", name=f"oT{b}{hh}")
                   for hh in range(HPC)] for b in range(B)]
            qT = [[qp.tile([P, T], bf16, tag=f"qT{b}{hh}", name=f"qT{b}{hh}")
                   for hh in range(HPC)] for b in range(B)]
            kT = [[qp.tile([P, T], bf16, tag=f"kT{b}{hh}", name=f"kT{b}{hh}")
                   for hh in range(HPC)] for b in range(B)]
            vt = [[qp.tile([P, NSC, D], bf16, tag=f"v{b}{hh}",
                           name=f"v{b}{hh}") for hh in range(HPC)]
                  for b in range(B)]

            # ---------------- emitters ----------------
            def rope(ps, dst, sl, cs, sn):
                # tcos = ps * [cos;cos]; tsw pre-swaps halves (tsw[0:64] =
                # q2*sin, tsw[64:128] = q1*sin) so reads align on base
                # partitions; muls on DVE (psum-capable), sub/add on gpsimd.
                tcos = rp.tile([P, TQ], f32, tag="tcos", name="tcos")
                tsw = rp.tile([P, TQ], f32, tag="tsw", name="tsw")
                nc.vector.tensor_mul(tcos[:], ps[:], cs)
                nc.vector.tensor_mul(tsw[0:HALF, :], ps[HALF:P, :], sn)
                nc.vector.tensor_mul(tsw[HALF:P, :], ps[0:HALF, :], sn)
                nc.gpsimd.tensor_sub(dst[0:HALF, sl], tcos[0:HALF, :],
                                     tsw[0:HALF, :])
                nc.gpsimd.tensor_add(dst[HALF:P, sl], tcos[HALF:P, :],
                                     tsw[HALF:P, :])

            def qk_group(b, ti, hh, w_w, dst):
                def emit():
                    sl = slice(ti * TQ, (ti + 1) * TQ)
                    xt = xts[(b, ti)]
                    hsl = slice(hh * D, (hh + 1) * D)
                    ps = ring_qk.tile([P, TQ], f32, tag="qk",
                                      name=f"qk{b}_{ti}_{hh}")
                    for ko in range(KO):
                        nc.tensor.matmul(ps[:], w_w(ko, hsl), xt[:, ko, :],
                                         start=(ko == 0), stop=(ko == KO - 1))
                    rope(ps, dst, sl, cs_sb[:, sl], sn_sb[:, sl])
                return (KO * TQ, emit)

            def v_group(b, ti, sub):
                def emit():
                    xt = xts[(b, ti)]
                    psv = ring_qk.tile([P, HPC * D], f32, tag="qk",
                                       name=f"v{b}_{ti}_{sub}")
                    for ko in range(KO):
                        nc.tensor.matmul(
                            psv[:], xt[:, ko, sub * P:(sub + 1) * P],
                            wv_sb[:, ko, :],
                            start=(ko == 0), stop=(ko == KO - 1))
                    tci = ti * (TQ // P) + sub
                    for hh in range(HPC):
                        nc.vector.tensor_copy(vt[b][hh][:, tci, :],
                                              psv[:, hh * D:(hh + 1) * D])
                return (KO * HPC * D, emit)

            def qkv_items(b):
                items = []
                wq_w = lambda ko, hsl: wq_g[ko // KG][:, ko % KG, hsl]
                wk_w = lambda ko, hsl: wk_sb[:, ko, hsl]
                for ti in range(NT):
                    for hh in range(HPC):
                        items.append(qk_group(b, ti, hh, wq_w, qT[b][hh]))
                    for hh in range(HPC):
                        items.append(qk_group(b, ti, hh, wk_w, kT[b][hh]))
                    for sub in range(TQ // P):
                        items.append(v_group(b, ti, sub))
                return items

            def attn_items(b):
                """Flat item list for batch b's attention: scores/exp chunks,
                AV+sum flushes lagging 2 slots, normalize lagging 3 more."""
                items = []
                norm_cd = []  # [countdown, (cost, fn)] pending normalizes

                def push(item):
                    items.append(item)
                    for e in norm_cd:
                        e[0] -= 1
                    while norm_cd and norm_cd[0][0] <= 0:
                        items.append(norm_cd.pop(0)[1])

                split = 0
                for hh in range(HPC):
                    if hh == 1:
                        split = len(items)
                    for ta in range(NTA):
                        nchunks = (ta + 1) * (TA // P)
                        st = {}

                        def chunk(s, hh=hh, ta=ta, st=st):
                            def emit():
                                diag = s >= ta * (TA // P)
                                t_lo = (s - ta * (TA // P)) * P if diag else 0
                                w = slice(t_lo, TA)
                                qsl = slice(ta * TA + t_lo, (ta + 1) * TA)
                                ps_sc = ring_sc.tile([P, TA], f32, tag="sc",
                                                     name=f"sc{b}{hh}{ta}_{s}")
                                nc.tensor.matmul(
                                    ps_sc[:, w],
                                    kT[b][hh][:, s * P:(s + 1) * P],
                                    qT[b][hh][:, qsl], start=True, stop=True)
                                pt = ptp.tile([P, TA], bf16, tag="pt",
                                              name=f"pt{b}{hh}{ta}_{s}")
                                nc.scalar.activation(
                                    pt[:, w], ps_sc[:, w],
                                    mybir.ActivationFunctionType.Exp,
                                    scale=SCALE)
                                if diag:  # mask the 128x128 triangle
                                    nc.vector.tensor_mul(
                                        pt[:, t_lo:t_lo + P],
                                        pt[:, t_lo:t_lo + P], mask_sb[:])
                                st[("pt", s)] = pt
                            return ((TA - (max(0, s - ta * (TA // P))) * P),
                                    emit)

                        def wsl(s, ta=ta):
                            diag = s >= ta * (TA // P)
                            t_lo = (s - ta * (TA // P)) * P if diag else 0
                            return slice(t_lo, TA)

                        def flush_av(s, hh=hh, ta=ta, st=st, nchunks=nchunks,
                                     wsl=wsl):
                            w = wsl(s)
                            if s == 0:
                                st["o"] = ring_acc.tile(
                                    [P, TA], f32, tag="o", name=f"o{b}{hh}{ta}")
                                st["sum"] = ring_sum.tile(
                                    [1, TA], f32, tag="sum",
                                    name=f"sum{b}{hh}{ta}")
                            nc.tensor.matmul(st["o"][:, w],
                                             vt[b][hh][:, s, :],
                                             st[("pt", s)][:, w],
                                             start=(s == 0),
                                             stop=(s == nchunks - 1))

                        def flush_sum(s, hh=hh, ta=ta, st=st, nchunks=nchunks,
                                      wsl=wsl):
                            w = wsl(s)
                            pt = st.pop(("pt", s))
                            nc.tensor.matmul(st["sum"][:, w], ones_col[:],
                                             pt[:, w], start=(s == 0),
                                             stop=(s == nchunks - 1))

                        def norm(hh=hh, ta=ta, st=st):
                            def emit():
                                recf = sp.tile([1, TA], f32, tag="recf",
                                               name="recf")
                                nc.vector.reciprocal_approx_fast(
                                    recf[:], st["sum"][:])
                                rec = sp.tile([1, TA], f32r, tag="rec",
                                              name="rec")
                                nc.vector.tensor_copy(rec[:], recf[:])
                                ps_bc = ring_sc.tile([P, TA], f32, tag="sc",
                                                     name=f"bc{b}{hh}{ta}")
                                nc.tensor.matmul(ps_bc[:], ones_row[:],
                                                 rec[:], start=True, stop=True)
                                bc_sb = sp.tile([P, TA], f32, tag="bc_sb",
                                                name="bc_sb")
                                nc.vector.tensor_copy(bc_sb[:], ps_bc[:])
                                # plain [d, t] oT write (contiguous DVE op);
                                # the out-projection shuffle happens in its
                                # strided stationary AP instead
                                nc.vector.tensor_mul(
                                    oT[b][hh][:, ta * TA:(ta + 1) * TA],
                                    st["o"][:], bc_sb[:])
                            return (TA, emit)

                        # quad weave: [c0..c3] [c4..c7] F(0..3) [c8..c11]
                        # F(4..7) ...  flush-quads emit AVx4 then sumx4 so
                        # consecutive matmuls share PSUM banks (fewer hops)
                        def flushquad(s0, fa=flush_av, fs=flush_sum, ta=ta):
                            def emit():
                                for s in range(s0, s0 + 4):
                                    fa(s)
                                for s in range(s0, s0 + 4):
                                    fs(s)
                            cost = sum(
                                2 * (TA - max(0, s - ta * (TA // P)) * P)
                                for s in range(s0, s0 + 4))
                            return (cost, emit)

                        for base in range(0, nchunks, 4):
                            for s in range(base, base + 4):
                                push(chunk(s))
                            if base >= 4:
                                push(flushquad(base - 4))
                        push(flushquad(nchunks - 4))
                        norm_cd.append([3, norm()])
                # leftover normalizes: paced into the next phase by the caller
                items.extend(e[1] for e in norm_cd)
                return items, split

            def outproj_items(b, half, heads):
                items = []
                for cpi in range(NTC):
                    for hh in heads:
                        def emit(cpi=cpi, hh=hh):
                            # contraction index k = (t%16)*128 + d; chunk j
                            # = t%16 needs oT columns {t = u*16+j} — strided
                            # stationary view of the plain [d, t] layout
                            ovw = oT[b][hh].rearrange("p (u j) -> p u j",
                                                      j=KO)
                            psy = ring_qk.tile([P, TC], f32, tag="qk",
                                               name=f"y{b}{hh}{cpi}")
                            for j in range(KO):
                                wt = wcs[(half, cpi, j // (KO // 2))]
                                nc.tensor.matmul(
                                    psy[:], ovw[:, :, j],
                                    wt[:, j % (KO // 2), :],
                                    start=(j == 0), stop=(j == KO - 1))
                            ysb = sp.tile([P, TC], f32, tag="ysb", name="ysb")
                            nc.vector.tensor_copy(ysb[:], psy[:])
                            nc.gpsimd.dma_start(
                                y.ap()[b, hh * D:(hh + 1) * D,
                                       cpi * TC:(cpi + 1) * TC], ysb[:])
                        items.append((KO * TC, emit))
                return items

            def merge(a_items, b_items, lead_a=0, bias=1.0):
                """Emit two independent item streams interleaved, pacing by
                cumulative estimated PE cost (each stream's internal order is
                preserved).  The first lead_a a-items are emitted before any
                b-item (emission order is dependency order: a b-item reading
                data produced by a deferred a-item must come after it)."""
                tot_a = sum(c for c, _ in a_items) or 1
                tot_b = sum(c for c, _ in b_items) or 1
                ca = cb = 0.0
                ia = ib = 0
                while ia < len(a_items) or ib < len(b_items):
                    take_a = ib >= len(b_items) or ia < lead_a or (
                        ia < len(a_items)
                        and ca / tot_a <= cb / (tot_b * bias))
                    if take_a:
                        c, fn = a_items[ia]; ia += 1; ca += c; fn()
                    else:
                        c, fn = b_items[ib]; ib += 1; cb += c; fn()

            def clump(items, k=2):
                # combine adjacent items so the merge inserts fewer
                # group<->chunk boundaries into the PE stream
                out = []
                for i in range(0, len(items), k):
                    grp = items[i:i + k]
                    fns = [f for _, f in grp]
                    out.append((sum(c for c, _ in grp),
                                lambda fns=fns: [f() for f in fns]))
                return out

            # ---------------- schedule ----------------
            for _, fn in qkv_items(0):              # P1
                fn()
            merge(attn_items(0)[0], clump(qkv_items(1)))   # P2
            # P3: attention(b1) with b0's out-projection as PE cover,
            # front-loaded (bias) so P4's wout copies land before P4 starts
            merge(attn_items(1)[0], outproj_items(0, 0, (0, 1)), bias=1.2)
            for _, fn in outproj_items(1, 1, (0, 1)):       # P4
                fn()

    nc.compile()
    return nc


_NC = None


def _get_nc():
    global _NC
    if _NC is None:
        _NC = _build()
    return _NC


def _host_tables():
    pos = np.arange(T, dtype=np.float32)[:, None]
    div = np.exp(np.arange(0, 2 * HALF, 2, dtype=np.float32)
                 * np.float32(-math.log(ROPE_BASE) / (2 * HALF)))
    ang = pos * div[None, :]
    cosv = np.cos(ang).astype(np.float32)   # [T, HALF]
    sinv = np.sin(ang).astype(np.float32)
    cosT = np.ascontiguousarray(cosv.T)     # [HALF, T]
    sinT = np.ascontiguousarray(sinv.T)
    cs2 = np.ascontiguousarray(
        np.concatenate([cosT, cosT], axis=0)).astype(npbf16)  # [P, T]
    sn1 = sinT.astype(npbf16)
    # triangle mask M[s, w] = 1 iff s <= w
    ww = np.arange(P)[None, :]
    ss = np.arange(P)[:, None]
    maskM = (ss <= ww).astype(npbf16)
    return cs2, sn1, maskM


def _make_in_maps(x, Wqkv, Wout):
    x = np.asarray(x, dtype=np.float32)
    Wqkv = np.asarray(Wqkv, dtype=np.float32)
    Wout = np.asarray(Wout, dtype=np.float32)
    assert x.shape == (B, T, C) and Wqkv.shape == (C, 3 * C) \
        and Wout.shape == (C, C)

    cs2, sn1, maskM = _host_tables()
    # xTt[b, ti, p, ko, u] = x[b, ti*TQ+u, ko*128+p]
    xTt = np.ascontiguousarray(
        x.reshape(B, NT, TQ, KO, P).transpose(0, 1, 4, 3, 2).astype(npbf16))
    # wout4[cpi, p, j, u] = Wout[j*128+p, cpi*TC+u]
    wout4 = np.ascontiguousarray(
        Wout.reshape(KO, P, NTC, TC).transpose(2, 1, 0, 3).astype(npbf16))

    in_maps = []
    for core in range(NCORES):
        h0 = core * HPC
        cols = slice(h0 * D, (h0 + HPC) * D)
        ws = []
        for part in range(3):
            w = Wqkv[:, part * C:(part + 1) * C][:, cols]  # [C, HPC*D]
            ws.append(np.ascontiguousarray(
                w.reshape(KO, P, HPC * D).transpose(1, 0, 2).astype(npbf16)))
        in_maps.append({
            "xTt": xTt,
            "wq": ws[0], "wk": ws[1], "wv": ws[2],
            "wout": wout4,
            "cs2": cs2, "sn1": sn1, "maskM": maskM,
        })
    return in_maps


def _run(x, Wqkv, Wout, trace=False):
    nc = _get_nc()
    in_maps = _make_in_maps(x, Wqkv, Wout)
    res = run_bass_kernel_spmd(nc, in_maps, core_ids=list(range(NCORES)),
                               trace=trace)
    out = np.empty((B, T, C), dtype=np.float32)
    for core in range(NCORES):
        out[:, core * HPC * D:(core + 1) * HPC * D, :] = \
            res.results[core]["y"]
    return out, res


def kernel(x, Wqkv, Wout):
    out, _ = _run(x, Wqkv, Wout)
    return out
